# revision 6
# baseline (speedup 1.0000x reference)
"""Trainium2 Bass kernel for a full transformer block (attention + 16x FFN).

Sharding: 8 cores = 4 batches x 2 zigzag row-sets. Each core computes K/V over
its batch's full 2048-token sequence and handles 8 query row-blocks of 128
rows chosen so every core's causal-extent multiset is (16,14,12,10,8,6,4,2)
key-blocks -- a perfectly uniform SPMD program, no collectives. FFN/LN are
token-parallel on the same rows. Diagonal-block masks are per-core data.

All matmuls run as float32r (fp32 storage, ~fp22 compute at bf16 speed).
Scores are computed transposed (S^T [keys, rows]) so softmax needs no
on-chip transposes; denominators come from a ones-column appended to V.
"""
from contextlib import ExitStack

import numpy as np

import concourse.bass as bass
import concourse.mybir as mybir
import concourse.tile as tile
from concourse import bacc
from concourse import bass_utils
from concourse.masks import make_identity

B, T, D, H, HD, FF = 4, 2048, 1024, 16, 64, 16 * 1024
TR = T // 2          # rows per core = 1024
NEG = -1e9
EPS = 1e-5
F32 = mybir.dt.float32
F32R = mybir.dt.float32r
AF = mybir.ActivationFunctionType

# rows prefix (in rows) that attends key-chunk kc, with slots sorted by
# descending extent E_j = 16-2j
N_KC = [128 * ((16 - kc + 1) // 2) for kc in range(16)]


def _r(ap):
    return ap.bitcast(F32R)


def blocks_for(parity):
    if parity == 1:
        return [15 - 2 * j for j in range(8)]
    return [14 - 2 * j for j in range(8)]


def _bcast_ap(src, parts):
    """AP replicated across `parts` partitions (partition-step 0)."""
    return bass.AP(tensor=src.tensor, offset=src.offset,
                   ap=[[0, parts]] + list(src.ap)[-1:])


def build_program():
    nc = bacc.Bacc("TRN2", target_bir_lowering=False, debug=False,
                   enable_asserts=False, num_devices=8)
    din = {}

    def d(name, shape):
        din[name] = nc.dram_tensor(name, list(shape), F32,
                                   kind="ExternalInput").ap()

    d("xT", (D, T)); d("xTq", (D, TR)); d("xr", (TR, D))
    d("wq", (D, D)); d("wk", (D, D)); d("wv", (D, D))
    d("bq", (D,)); d("bk", (D,)); d("bv", (D,))
    d("wo", (D, D)); d("bo", (D,))
    d("g1v", (D,)); d("b1v", (D,)); d("g2v", (D,)); d("b2v", (D,))
    d("w1", (D, FF)); d("b1t", (128, 128)); d("w2", (FF, D)); d("b2", (D,))
    d("maskAB", (128, 256)); d("onesd", (512,))
    out_d = nc.dram_tensor("out", [TR, D], F32, kind="ExternalOutput").ap()

    with tile.TileContext(nc) as tc:
        _build(tc, nc, din, out_d)
    nc.compile()
    return nc


def _build(tc, nc, din, out_d):
    with ExitStack() as ctx:
        consts = ctx.enter_context(tc.tile_pool(name="consts", bufs=1))
        ident = consts.tile([128, 128], F32)
        make_identity(nc, ident)
        ones = consts.tile([1, 512], F32R)
        nc.sync.dma_start(ones, _r(din["onesd"][None, :]))
        eps_t = consts.tile([128, 1], F32)
        nc.vector.memset(eps_t, EPS)
        maskab = consts.tile([128, 256], F32)
        nc.sync.dma_start(maskab, din["maskAB"][:, :])
        b1t_sb = consts.tile([128, 128], F32)
        nc.sync.dma_start(b1t_sb, din["b1t"][:, :])
        r_all = consts.tile([16, TR], F32)
        rrec = consts.tile([16, TR], F32)

        def vec1(pool, name):
            t = pool.tile([1, D], F32R, name=f"sb_{name}", tag=f"sb_{name}")
            nc.sync.dma_start(t, _r(din[name][None, :]))
            return t

        def vbc(pool, name):
            t = pool.tile([128, D], F32, name=f"bc_{name}", tag=f"bc_{name}")
            nc.gpsimd.dma_start(t, _bcast_ap(din[name][None, :], 128))
            return t

        ot_es = ctx.enter_context(ExitStack())
        ot_pool = ot_es.enter_context(
            tc.tile_pool(name="otp", bufs=1, side="right"))
        ot = [ot_pool.tile([128, TR], F32R, name=f"ot{p}", tag=f"ot{p}")
              for p in range(8)]

        with ExitStack() as qs:
            qt_pool = qs.enter_context(tc.tile_pool(name="qtp", bufs=1))
            qt = [qt_pool.tile([128, TR], F32R, name=f"qt{p}", tag=f"qt{p}")
                  for p in range(8)]

            # ---------------- Phase Q: all q projections ----------------
            with tc.tile_pool(name="phq", bufs=1) as phq, \
                 tc.tile_pool(name="phq_w", bufs=6) as phq_w, \
                 tc.tile_pool(name="proj_ps", bufs=2, space="PSUM") as proj_ps:
                bq_sb = vec1(phq, "bq")
                xtq = {}
                for dc in range(8):
                    for nh in range(2):
                        t = phq.tile([128, 512], F32R, name=f"xtq{dc}_{nh}",
                                     tag=f"xtq{dc}_{nh}")
                        nc.sync.dma_start(
                            t, _r(din["xTq"][128 * dc:128 * dc + 128,
                                             512 * nh:512 * nh + 512]))
                        xtq[dc, nh] = t
                for p in range(8):
                    wqt = []
                    for dc in range(8):
                        t = phq_w.tile([128, 128], F32R, name="wqt", tag="wq",
                                       bufs=10)
                        nc.sync.dma_start(
                            t, _r(din["wq"][128 * dc:128 * dc + 128,
                                            128 * p:128 * p + 128]))
                        wqt.append(t)
                    for nh in range(2):
                        ps = proj_ps.tile([128, 512], F32, name="qps",
                                          tag="proj")
                        for dc in range(8):
                            nc.tensor.matmul(ps, wqt[dc], xtq[dc, nh],
                                             start=(dc == 0), stop=False,
                                             skip_group_check=True)
                        nc.tensor.matmul(
                            ps, bq_sb[0:1, 128 * p:128 * p + 128],
                            ones[0:1, :], start=False, stop=True,
                            skip_group_check=True)
                        nc.vector.tensor_copy(
                            qt[p][:, 512 * nh:512 * nh + 512], ps)

            # ---------------- attention ----------------
            with ExitStack() as ats:
                att = ats.enter_context(tc.tile_pool(name="att", bufs=1))
                kt_pool = ats.enter_context(tc.tile_pool(name="ktp", bufs=1))
                wk_pool = ats.enter_context(tc.tile_pool(name="wkp", bufs=4))
                xts_pool = ats.enter_context(tc.tile_pool(name="xts", bufs=1))
                pt_pool = ats.enter_context(tc.tile_pool(name="ptp", bufs=1))
                stage_pool = ats.enter_context(
                    tc.tile_pool(name="stage", bufs=2))
                s_ps_pool = ats.enter_context(
                    tc.tile_pool(name="s_ps", bufs=2, space="PSUM"))
                av_ps_pool = ats.enter_context(
                    tc.tile_pool(name="av_ps", bufs=2, space="PSUM"))
                pj_ps_pool = ats.enter_context(
                    tc.tile_pool(name="pj_ps", bufs=2, space="PSUM"))

                bk_sb = vec1(att, "bk")
                bv_sb = vec1(att, "bv")
                v8 = {}

                def load_xts(dc, n4):
                    t = xts_pool.tile([128, 512], F32R, name="xts",
                                      tag=f"xs{dc}")
                    nc.sync.dma_start(
                        t, _r(din["xT"][128 * dc:128 * dc + 128,
                                        512 * n4:512 * n4 + 512]))
                    return t

                def kt_mms(p, ps, xts, n4):
                    for dc in range(8):
                        nc.tensor.matmul(ps, wkt[dc], xts[dc],
                                         start=(dc == 0), stop=False,
                                         skip_group_check=True)
                    nc.tensor.matmul(
                        ps, bk_sb[0:1, 128 * p:128 * p + 128],
                        ones[0:1, :], start=False, stop=True,
                        skip_group_check=True)

                for p in range(8):
                    G = p // 4
                    wkt = []
                    for dc in range(8):
                        t = wk_pool.tile([128, 128], F32R, name="wkt", tag="wk",
                                         bufs=10)
                        nc.sync.dma_start(
                            t, _r(din["wk"][128 * dc:128 * dc + 128,
                                            128 * p:128 * p + 128]))
                        wkt.append(t)
                    kt = kt_pool.tile([128, T], F32R, name="kt", tag="kt")
                    if p % 4 == 0:
                        # V-group projection shares the xT stream with kT
                        with tc.tile_pool(name="wvp", bufs=1) as wvp:
                            wvt = []
                            for dc in range(8):
                                t = wvp.tile([128, 512], F32R, name="wvt",
                                             tag=f"wv{dc}")
                                nc.sync.dma_start(
                                    t, _r(din["wv"][128 * dc:128 * dc + 128,
                                                    512 * G:512 * G + 512]))
                                wvt.append(t)
                            for n4 in range(4):
                                xts = [load_xts(dc, n4) for dc in range(8)]
                                for kcl in range(4):
                                    kc = 4 * n4 + kcl
                                    ps = pj_ps_pool.tile([128, 512], F32,
                                                         name="vps", tag="pj")
                                    for dc in range(8):
                                        nc.tensor.matmul(
                                            ps,
                                            xts[dc][:, 128 * kcl:
                                                    128 * kcl + 128],
                                            wvt[dc], start=(dc == 0),
                                            stop=False, skip_group_check=True)
                                    nc.tensor.matmul(
                                        ps, ones[0:1, 0:128],
                                        bv_sb[0:1, 512 * G:512 * G + 512],
                                        start=False, stop=True,
                                        skip_group_check=True)
                                    vt = att.tile([128, 8, 65], F32R,
                                                  name="v8", tag=f"v8_{kc}")
                                    nc.vector.tensor_copy(
                                        vt[:, :, 0:64],
                                        ps.rearrange("p (h e) -> p h e", h=8))
                                    nc.gpsimd.dma_start(
                                        vt[:, :, 64],
                                        _r(_bcast_ap(din["onesd"][None, 0:8],
                                                     128)))
                                    v8[kc] = vt
                                kps = pj_ps_pool.tile([128, 512], F32,
                                                      name="kps", tag="pj")
                                kt_mms(p, kps, xts, n4)
                                nc.vector.tensor_copy(
                                    kt[:, 512 * n4:512 * n4 + 512], kps)
                    else:
                        for n4 in range(4):
                            xts = [load_xts(dc, n4) for dc in range(8)]
                            kps = pj_ps_pool.tile([128, 512], F32, name="kps",
                                                  tag="pj")
                            kt_mms(p, kps, xts, n4)
                            nc.vector.tensor_copy(
                                kt[:, 512 * n4:512 * n4 + 512], kps)

                    for h01 in range(2):
                        h = 2 * p + h01
                        hb = 64 * h01
                        gh = h % 8
                        pts = []
                        for kc in range(16):
                            N = N_KC[kc]
                            sps = s_ps_pool.tile([128, 1024], F32, name="sps",
                                                 tag="s")
                            for half in range((N + 511) // 512):
                                n0 = 512 * half
                                n1 = min(N, n0 + 512)
                                nc.tensor.matmul(
                                    sps[:, n0:n1],
                                    kt[hb:hb + 64,
                                       128 * kc:128 * kc + 128],
                                    qt[p][hb:hb + 64, n0:n1],
                                    start=True, stop=True,
                                    skip_group_check=True)
                            if kc % 2 == 0:
                                j, msl = (14 - kc) // 2, maskab[:, 0:128]
                            else:
                                j, msl = (15 - kc) // 2, maskab[:, 128:256]
                            nc.vector.tensor_add(
                                sps[:, 128 * j:128 * j + 128],
                                sps[:, 128 * j:128 * j + 128], msl)
                            pt = pt_pool.tile([128, N], F32R, name="pt",
                                              tag=f"pt{kc}")
                            nc.scalar.activation(pt, sps[:, 0:N], AF.Exp,
                                                 scale=0.125)
                            pts.append(pt)
                        for rg in range(2):
                            kcs = range(16) if rg == 0 else range(8)
                            last = 15 if rg == 0 else 7
                            av = av_ps_pool.tile([65, 512], F32, name="av",
                                                 tag="av")
                            for kc in kcs:
                                w = min(512, N_KC[kc] - 512 * rg)
                                nc.tensor.matmul(
                                    av[:, 0:w], v8[kc][:, gh, :],
                                    pts[kc][:, 512 * rg:512 * rg + w],
                                    start=(kc == 0), stop=(kc == last),
                                    skip_group_check=True)
                            stg = stage_pool.tile([65, 512], F32, name="stg",
                                                  tag="stg")
                            nc.vector.tensor_copy(stg, av)
                            nc.sync.dma_start(
                                ot[p][hb:hb + 64, 512 * rg:512 * rg + 512],
                                _r(stg[0:64, :]))
                            nc.sync.dma_start(
                                r_all[h:h + 1, 512 * rg:512 * rg + 512],
                                stg[64:65, :])

            # normalize OT by 1/rowsum
            nc.vector.reciprocal(rrec, r_all)
            with tc.tile_pool(name="rbp", bufs=2) as rbp, \
                 tc.tile_pool(name="rbd", bufs=1, space="DRAM") as rbd:
                rdram = rbd.tile([16, TR], F32, name="rdram")
                nc.sync.dma_start(rdram, rrec)
                for p in range(8):
                    rb = rbp.tile([128, TR], F32, name="rb", tag="rb")
                    for h01 in range(2):
                        nc.gpsimd.dma_start(
                            rb[64 * h01:64 * h01 + 64, :],
                            _bcast_ap(rdram[2 * p + h01:2 * p + h01 + 1, :],
                                      64))
                    nc.vector.tensor_mul(ot[p], ot[p], rb)
        # qt released here

        o1_pool = ctx.enter_context(tc.tile_pool(name="o1p", bufs=1))
        out1 = [o1_pool.tile([128, D], F32, name=f"o1_{rc}", tag=f"o1_{rc}")
                for rc in range(8)]
        out1T = [o1_pool.tile([128, TR], F32R, name=f"o1T_{dc}",
                              tag=f"o1T_{dc}") for dc in range(8)]

        # ---------------- Wo + LN1 + transpose ----------------
        with tc.tile_pool(name="wop", bufs=1) as wop, \
             tc.tile_pool(name="lnp", bufs=3) as lnp, \
             tc.tile_pool(name="wo_ps", bufs=4, space="PSUM") as wo_ps, \
             tc.tile_pool(name="tr_ps", bufs=4, space="PSUM") as tr_ps:
            bo_sb = vec1(wop, "bo")
            g1bc, b1bc = vbc(wop, "g1v"), vbc(wop, "b1v")
            wot = []
            for pc in range(8):
                t = wop.tile([128, D], F32R, name="wot", tag=f"wo{pc}")
                nc.sync.dma_start(t, _r(din["wo"][128 * pc:128 * pc + 128, :]))
                wot.append(t)
            for rc in range(8):
                xrt = lnp.tile([128, D], F32, name="xrt", tag="xr")
                nc.sync.dma_start(xrt, din["xr"][128 * rc:128 * rc + 128, :])
                y = lnp.tile([128, D], F32, name="y", tag="y")
                for nh in range(2):
                    ps = wo_ps.tile([128, 512], F32, name="wops", tag="wo")
                    for pc in range(8):
                        nc.tensor.matmul(
                            ps, ot[pc][:, 128 * rc:128 * rc + 128],
                            wot[pc][:, 512 * nh:512 * nh + 512],
                            start=(pc == 0), stop=False, skip_group_check=True)
                    nc.tensor.matmul(ps, ones[0:1, 0:128],
                                     bo_sb[0:1, 512 * nh:512 * nh + 512],
                                     start=False, stop=True,
                                     skip_group_check=True)
                    nc.vector.tensor_add(y[:, 512 * nh:512 * nh + 512], ps,
                                         xrt[:, 512 * nh:512 * nh + 512])
                _layernorm(nc, lnp, y, out1[rc], g1bc, b1bc, eps_t)
                for dc in range(8):
                    tp = tr_ps.tile([128, 128], F32, name="trp", tag="tr")
                    nc.tensor.transpose(
                        tp, out1[rc][:, 128 * dc:128 * dc + 128], ident)
                    nc.vector.tensor_copy(
                        out1T[dc][:, 128 * rc:128 * rc + 128], tp)
        ot_es.close()

        # ---------------- FFN ----------------
        with tc.tile_pool(name="w1p", bufs=5) as w1p, \
             tc.tile_pool(name="w2p", bufs=9) as w2p, \
             tc.tile_pool(name="h1p", bufs=9) as h1p, \
             tc.tile_pool(name="ffb", bufs=1) as ffb, \
             tc.tile_pool(name="h1_ps", bufs=2, space="PSUM") as h1_ps, \
             tc.tile_pool(name="w2_ps", bufs=2, space="PSUM") as w2_ps:
            b2_sb = vec1(ffb, "b2")
            w1r = din["w1"].rearrange("(o p) f -> p o f", p=128)
            for sb in range(16):
                h1s, w2s = [], []
                for fp in range(4):     # ffc pairs
                    ffc0 = 8 * sb + 2 * fp
                    w1t = w1p.tile([128, 8, 256], F32R, name="w1t", tag="w1")
                    nc.sync.dma_start(
                        w1t, _r(w1r[:, :, 128 * ffc0:128 * ffc0 + 256]))
                    for fo in range(2):
                        ffc = ffc0 + fo
                        hp = h1_ps.tile([128, 1024], F32, name="hps",
                                        tag="h1")
                        for dc in range(8):
                            for nh in range(2):
                                nc.tensor.matmul(
                                    hp[:, 512 * nh:512 * nh + 512],
                                    w1t[:, dc, 128 * fo:128 * fo + 128],
                                    out1T[dc][:, 512 * nh:512 * nh + 512],
                                    start=(dc == 0), stop=(dc == 7),
                                    skip_group_check=True)
                        h1 = h1p.tile([128, 1024], F32R, name="h1", tag="h1")
                        nc.scalar.activation(h1, hp, AF.Relu,
                                             bias=b1t_sb[:, ffc:ffc + 1],
                                             scale=1.0)
                        h1s.append(h1)
                        w2t = w2p.tile([128, D], F32R, name="w2t", tag="w2")
                        nc.sync.dma_start(
                            w2t, _r(din["w2"][128 * ffc:128 * ffc + 128, :]))
                        w2s.append(w2t)
                for rc in range(8):
                    wp = w2_ps.tile([128, 1024], F32, name="wps", tag="w2")
                    for f8 in range(8):
                        for nh in range(2):
                            st = (f8 == 7) and sb != 0
                            nc.tensor.matmul(
                                wp[:, 512 * nh:512 * nh + 512],
                                h1s[f8][:, 128 * rc:128 * rc + 128],
                                w2s[f8][:, 512 * nh:512 * nh + 512],
                                start=(f8 == 0), stop=st,
                                skip_group_check=True)
                    if sb == 0:
                        for nh in range(2):
                            nc.tensor.matmul(
                                wp[:, 512 * nh:512 * nh + 512],
                                ones[0:1, 0:128],
                                b2_sb[0:1, 512 * nh:512 * nh + 512],
                                start=False, stop=True, skip_group_check=True)
                    nc.vector.tensor_add(out1[rc], out1[rc], wp)

        # ---------------- LN2 + output ----------------
        with tc.tile_pool(name="ln2p", bufs=3) as ln2p:
            g2bc, b2bc = vbc(ln2p, "g2v"), vbc(ln2p, "b2v")
            for rc in range(8):
                o2 = ln2p.tile([128, D], F32, name="o2", tag="o2")
                _layernorm(nc, ln2p, out1[rc], o2, g2bc, b2bc, eps_t)
                nc.sync.dma_start(out_d[128 * rc:128 * rc + 128, :], o2)


def _layernorm(nc, pool, y, out, gbc, bbc, eps_t):
    stats = pool.tile([128, 2, 6], F32, name="lnst", tag="lnst")
    nc.vector.bn_stats(out=stats[:, 0, :], in_=y[:, 0:512])
    nc.vector.bn_stats(out=stats[:, 1, :], in_=y[:, 512:1024])
    mv = pool.tile([128, 2], F32, name="lnmv", tag="lnmv")
    nc.vector.bn_aggr(out=mv, in_=stats)
    istd = pool.tile([128, 1], F32, name="lnis", tag="lnis")
    nc.scalar.activation(istd, mv[:, 1:2], AF.Sqrt, bias=eps_t, scale=1.0)
    nc.vector.reciprocal(istd, istd)
    nc.vector.tensor_scalar(out, y, mv[:, 0:1], istd,
                            mybir.AluOpType.subtract, mybir.AluOpType.mult)
    nc.vector.tensor_mul(out, out, gbc)
    nc.vector.tensor_add(out, out, bbc)


# ---------------------------------------------------------------------------
# host side
# ---------------------------------------------------------------------------

def make_masks(parity):
    tc_ = np.where(np.arange(128)[:, None] <= np.arange(128)[None, :],
                   np.float32(0), np.float32(NEG))
    if parity == 1:
        maskA = np.zeros((128, 128), np.float32)
        maskB = tc_
    else:
        maskA = tc_
        maskB = np.full((128, 128), NEG, np.float32)
    return np.concatenate([maskA, maskB], axis=1)


def prep_inputs(x, Wq, bq, Wk, bk, Wv, bv, Wo, bo, ln1_g, ln1_b,
                W1, b1, W2, b2, ln2_g, ln2_b):
    c = np.ascontiguousarray
    f = np.float32
    shared = {
        "wq": c(np.transpose(Wq, (1, 0, 2)).reshape(D, D).astype(f)),
        "wk": c(np.transpose(Wk, (1, 0, 2)).reshape(D, D).astype(f)),
        "wv": c(np.transpose(Wv, (1, 0, 2)).reshape(D, D).astype(f)),
        "bq": c(np.asarray(bq).reshape(-1).astype(f)),
        "bk": c(np.asarray(bk).reshape(-1).astype(f)),
        "bv": c(np.asarray(bv).reshape(-1).astype(f)),
        "wo": c(np.asarray(Wo).astype(f)), "bo": c(np.asarray(bo).astype(f)),
        "g1v": c(np.asarray(ln1_g).astype(f)),
        "b1v": c(np.asarray(ln1_b).astype(f)),
        "g2v": c(np.asarray(ln2_g).astype(f)),
        "b2v": c(np.asarray(ln2_b).astype(f)),
        "w1": c(np.asarray(W1).astype(f)),
        "b1t": c(np.asarray(b1).reshape(128, 128).T.astype(f)),
        "w2": c(np.asarray(W2).astype(f)),
        "b2": c(np.asarray(b2).astype(f)),
        "onesd": np.ones(512, f),
    }
    in_maps, rows_list = [], []
    for b in range(B):
        for parity in (0, 1):
            rows = np.concatenate(
                [np.arange(128 * g, 128 * g + 128)
                 for g in blocks_for(parity)])
            rows_list.append((b, rows))
            xb = np.asarray(x[b], f)
            m = dict(shared)
            m["xT"] = c(xb.T)
            m["xTq"] = c(xb[rows].T)
            m["xr"] = c(xb[rows])
            m["maskAB"] = make_masks(parity)
            in_maps.append(m)
    return in_maps, rows_list


_NC_CACHE = []


def _get_nc():
    if not _NC_CACHE:
        _NC_CACHE.append(build_program())
    return _NC_CACHE[0]


def kernel(**inputs):
    inputs = {k: np.asarray(v) for k, v in inputs.items()}
    in_maps, rows_list = prep_inputs(**inputs)
    nc = _get_nc()
    res = bass_utils.run_bass_kernel_spmd(nc, in_maps, core_ids=list(range(8)))
    out = np.zeros((B, T, D), np.float32)
    for i, (b, rows) in enumerate(rows_list):
        out[b][rows] = res.results[i]["out"]
    return out
